# revision 8
# baseline (speedup 1.0000x reference)
"""DBRX attention block as a Bass/Tile kernel for 8 Trainium2 NeuronCores.

Shapes (hardcoded): B=2, S=2048, HID=2048, NH=16, NKV=4, HD=128, clip +-8,
rope theta 5e5.  Sharding: DP2 x TP4, core c = (b=c//4, g=c%4): batch b,
q-heads 4g..4g+3, kv-head g. Host sums the 4 TP partials per batch.

v2 vs baseline:
- all matmul operands bf16 (host-converted): half DMA, 2x DVE elementwise.
- softmax denominators: exp'd score tiles are accumulated on DVE (bf16 2x)
  and reduced with ONE ones-matmul per (j,head) instead of one per k-block.
- rotate-half via partition-offset DVE reads (no rotm matmul, no PSUM trip).
- V is produced directly in [t, d] layout by swapping lhsT/rhs in its
  projection matmul (free=128 bf16), eliminating PE transposes + copies.
- out projection DMAs straight from PSUM (no SBUF staging copy).
- software-pipelined emission: phase1(tb) | attention(tb-1) | outproj(tb-2)
  round-robin so Act-limited attention overlaps PE-dense GEMM stretches.
"""

import math
from contextlib import ExitStack

import numpy as np

import concourse.bacc as bacc
import concourse.bass as bass
import concourse.mybir as mybir
import concourse.tile as tile
from concourse.bass_utils import run_bass_kernel_spmd

P = 128
B, S, HID = 2, 2048, 2048
NH, NKV, HD = 16, 4, 128
CLIP = 8.0
ROPE_THETA = 500000.0
NQ = NH // NKV        # q heads per core = 4
KC = HID // P         # 16 contraction chunks
TB = 512              # t-block
NTB = S // TB         # 4
QB = 512
NQB = S // QB
NKB = S // P          # 16
NCORES = 8

BF = mybir.dt.bfloat16
F32 = mybir.dt.float32


def build_nc(reps: int = 1):
    nc = bacc.Bacc()

    hT = nc.dram_tensor("hT", [HID, S], BF, kind="ExternalInput")
    wqkvT = nc.dram_tensor("wqkvT", [HID, 6 * P], BF, kind="ExternalInput")
    woutT = nc.dram_tensor("woutT", [NQ * P, HID], BF, kind="ExternalInput")
    cosT = nc.dram_tensor("cosT", [P, S], BF, kind="ExternalInput")
    sinT = nc.dram_tensor("sinT", [P, S], F32, kind="ExternalInput")
    out = nc.dram_tensor("out", [S, HID], F32, kind="ExternalOutput")

    hT3 = hT.rearrange("(kc p) t -> p kc t", p=P)          # [128, 16, 2048]
    wq3 = wqkvT.rearrange("(kc p) o -> p kc o", p=P)       # [128, 16, 768]
    wo3 = woutT.rearrange("(c p) o -> p c o", p=P)         # [128, 4, 2048]
    out3 = out.rearrange("(tc p) o -> p tc o", p=P)        # [128, 16, 2048]

    with TileCtx(nc, reps) as tc:
        emit_body(nc, tc, hT3, wq3, wo3, cosT, sinT, out3)

    nc.compile()
    return nc


class TileCtx:
    def __init__(self, nc, reps):
        self.nc = nc
        self.reps = reps
        self.tc = tile.TileContext(nc)
        self.loop = None

    def __enter__(self):
        tc = self.tc.__enter__()
        if self.reps > 1:
            self.loop = tc.For_i(0, self.reps, 1)
            self.loop.__enter__()
        return tc

    def __exit__(self, *a):
        if self.loop is not None:
            self.loop.__exit__(*a)
        return self.tc.__exit__(*a)


def emit_body(nc, tc, hT3, wq3, wo3, cosT, sinT, out3):
    with ExitStack() as ctx:
        persist = ctx.enter_context(tc.tile_pool(name="persist", bufs=1))
        qkv = persist.tile([P, 5, S], BF)       # q0..q3, k  (qkv^T layout)
        V = persist.tile([P, NKB, HD], BF)      # [t_local, kb, d]
        attnT = persist.tile([P, NQ, S], BF)
        h_sb = persist.tile([P, KC, S], BF)
        wq_sb = persist.tile([P, KC, 6 * P], BF)
        wout_sb = persist.tile([P, NQ, HID], BF)
        cos_sb = persist.tile([P, S], BF)
        sin_sb = persist.tile([P, S], F32)
        ones = persist.tile([P, P], BF)
        masks = persist.tile([P, P], BF)
        rotm = persist.tile([P, P], BF)     # rotate-half permutation (lhsT)
        setup_f32 = persist.tile([P, P], F32)

        # ---- prologue DMAs ----
        for kc in range(KC):
            nc.sync.dma_start(out=wq_sb[:, kc, NQ * P:],
                              in_=wq3[:, kc, NQ * P:])      # k+v cols
            nc.sync.dma_start(out=h_sb[:, kc, 0:TB], in_=hT3[:, kc, 0:TB])
            nc.sync.dma_start(out=wq_sb[:, kc, :NQ * P],
                              in_=wq3[:, kc, :NQ * P])      # q cols
        nc.gpsimd.dma_start(out=cos_sb, in_=cosT[:, :])
        nc.gpsimd.dma_start(out=sin_sb, in_=sinT[:, :])
        nc.gpsimd.dma_start(out=wout_sb, in_=wo3)

        nc.vector.memset(setup_f32, 1.0)
        nc.vector.tensor_copy(out=ones, in_=setup_f32)
        nc.gpsimd.memset(setup_f32, 1.0)
        nc.gpsimd.affine_select(      # keep 1.0 where q_local >= k_local
            out=setup_f32, in_=setup_f32,
            compare_op=mybir.AluOpType.is_ge, fill=0.0,
            base=0, channel_multiplier=-1, pattern=[[1, P]])
        nc.vector.tensor_copy(out=masks, in_=setup_f32)
        # rotm[p, x] = 1 at x = (p+64) % 128
        nc.gpsimd.memset(setup_f32, 0.0)
        nc.gpsimd.affine_select(
            out=setup_f32, in_=setup_f32,
            compare_op=mybir.AluOpType.not_equal, fill=1.0,
            base=64, channel_multiplier=1, pattern=[[-1, P]])
        nc.gpsimd.affine_select(
            out=setup_f32, in_=setup_f32,
            compare_op=mybir.AluOpType.not_equal, fill=1.0,
            base=-64, channel_multiplier=1, pattern=[[-1, P]])
        nc.vector.tensor_copy(out=rotm, in_=setup_f32)

        ps1 = ctx.enter_context(
            tc.tile_pool(name="ps1", bufs=2, space="PSUM"))
        pss = ctx.enter_context(
            tc.tile_pool(name="pss", bufs=3, space="PSUM"))
        pso = ctx.enter_context(
            tc.tile_pool(name="pso", bufs=2, space="PSUM"))
        psf = ps1  # out-proj shares the phase-1 pool (both short-lived)
        rope_p = ctx.enter_context(tc.tile_pool(name="rope", bufs=2))
        pt_p = ctx.enter_context(tc.tile_pool(name="pt", bufs=3))
        dn_p = ctx.enter_context(tc.tile_pool(name="dn", bufs=2))
        nrm_p = ctx.enter_context(tc.tile_pool(name="nrm", bufs=2))
        out_p = ctx.enter_context(tc.tile_pool(name="outp", bufs=3))

        inv_sqrt_hd = 1.0 / math.sqrt(HD)
        kT = qkv[:, NQ, :]

        def rope_block(oc, tb):
            sl = slice(tb * TB, (tb + 1) * TB)
            ch = qkv[:, oc, sl]
            rps = ps1.tile([P, TB], F32, tag="ps")
            nc.tensor.matmul(rps, rotm, ch, start=True, stop=True)
            t1 = rope_p.tile([P, TB], BF, tag="t1")
            nc.vector.tensor_mul(t1, rps, sin_sb[:, sl])  # sin sign-folded
            t2 = rope_p.tile([P, TB], BF, tag="t2")
            nc.gpsimd.tensor_mul(t2, ch, cos_sb[:, sl])
            nc.vector.tensor_add(ch, t1, t2)

        def g1(tb):
            """phase 1 for t-block tb: k, q0..q3 (clip+rope), then V chunks."""
            if tb + 1 < NTB:  # prefetch next h t-block
                for kc in range(KC):
                    nc.sync.dma_start(
                        out=h_sb[:, kc, (tb + 1) * TB:(tb + 2) * TB],
                        in_=hT3[:, kc, (tb + 1) * TB:(tb + 2) * TB])
            sl = slice(tb * TB, (tb + 1) * TB)
            for oc in [NQ, 0, 1, 2, 3]:
                ps = ps1.tile([P, TB], F32, tag="ps")
                for kc in range(KC):
                    nc.tensor.matmul(
                        ps, wq_sb[:, kc, oc * P:(oc + 1) * P],
                        h_sb[:, kc, sl],
                        start=(kc == 0), stop=(kc == KC - 1))
                nc.vector.tensor_scalar(
                    out=qkv[:, oc, sl], in0=ps,
                    scalar1=CLIP, scalar2=-CLIP,
                    op0=mybir.AluOpType.min, op1=mybir.AluOpType.max)
                rope_block(oc, tb)
                yield
            for i in range(TB // P):   # V directly in [t, d] layout
                kb = tb * (TB // P) + i
                ps = ps1.tile([P, TB], F32, tag="ps")
                for kc in range(KC):
                    nc.tensor.matmul(
                        ps[:, :HD], h_sb[:, kc, kb * P:(kb + 1) * P],
                        wq_sb[:, kc, 5 * P:6 * P],
                        start=(kc == 0), stop=(kc == KC - 1))
                nc.vector.tensor_scalar(
                    out=V[:, kb, :], in0=ps[:, :HD],
                    scalar1=CLIP, scalar2=-CLIP,
                    op0=mybir.AluOpType.min, op1=mybir.AluOpType.max)
                yield

        def ga(j):
            """attention for q-block j (all 4 heads), S^T layout."""
            nk = (j + 1) * (QB // P)
            for h in range(NQ):
                qT = qkv[:, h, :]
                ps_o = pso.tile([P, QB], F32, tag="ps_o")
                acc = dn_p.tile([P, QB], BF, tag="acc")
                pend = None   # (kb, q0, w, p_t) with exp in flight

                def flush(pend):
                    kb, q0, w, p_t = pend
                    nc.tensor.matmul(
                        ps_o[:, q0:], V[:, kb, :], p_t[:, :w],
                        start=(kb == 0), stop=(kb == nk - 1))
                    if kb == 0:
                        nc.vector.tensor_copy(out=acc, in_=p_t)
                    else:
                        nc.vector.tensor_add(
                            acc[:, q0:], acc[:, q0:], p_t[:, :w])

                for kb in range(nk):
                    r = kb - j * (QB // P)
                    q0 = max(0, P * r)
                    w = QB - q0
                    ps_s = pss.tile([P, QB], F32, tag="ps_s")
                    nc.tensor.matmul(
                        ps_s[:, :w], kT[:, kb * P:(kb + 1) * P],
                        qT[:, j * QB + q0:(j + 1) * QB],
                        start=True, stop=True)
                    p_t = pt_p.tile([P, QB], BF, tag="pt")
                    nc.scalar.activation(
                        p_t[:, :w], ps_s[:, :w],
                        mybir.ActivationFunctionType.Exp,
                        scale=inv_sqrt_hd)
                    if r >= 0:
                        nc.vector.tensor_mul(p_t[:, :P], p_t[:, :P], masks)
                    if pend is not None:
                        flush(pend)   # PV of kb-1 lands after scores(kb)
                    pend = (kb, q0, w, p_t)
                    yield
                flush(pend)
                ps_d = pss.tile([P, QB], F32, tag="psd", bufs=1)
                nc.tensor.matmul(ps_d, ones, acc, start=True, stop=True)
                recip = nrm_p.tile([P, QB], F32, tag="recip")
                nc.vector.reciprocal(recip, ps_d)
                nc.vector.tensor_mul(
                    attnT[:, h, j * QB:(j + 1) * QB], ps_o, recip)
                yield

        def go(j):
            """out projection for token rows of q-block j."""
            for tci in range(j * (QB // P), (j + 1) * (QB // P)):
                for ob in range(HID // TB):
                    ps = psf.tile([P, TB], F32, tag="ps")
                    for c in range(NQ):
                        nc.tensor.matmul(
                            ps, attnT[:, c, tci * P:(tci + 1) * P],
                            wout_sb[:, c, ob * TB:(ob + 1) * TB],
                            start=(c == 0), stop=(c == NQ - 1))
                    o_row = out_p.tile([P, TB], F32, tag="orow")
                    if ob % 2 == 0:   # split PSUM-drain between Act and DVE
                        nc.scalar.activation(
                            o_row, ps, mybir.ActivationFunctionType.Copy)
                    else:
                        nc.vector.tensor_copy(out=o_row, in_=ps)
                    nc.sync.dma_start(
                        out=out3[:, tci, ob * TB:(ob + 1) * TB], in_=o_row)
                    yield

        # ---- software-pipelined emission ----
        segments = []
        for tb in range(NTB):
            gens = [(g1(tb), 1)]
            if tb >= 1:
                gens.append((ga(tb - 1), 3))
            if tb >= 2:
                gens.append((go(tb - 2), 1))
            segments.append(gens)
        segments.append([(ga(NTB - 1), 3), (go(NTB - 2), 1)])
        segments.append([(go(NTB - 1), 1)])

        # attention steps are ~6x cheaper on PE than a phase-1/out-proj
        # unit and the Act engine must stay fed, so pull ga 3x per round
        for gens in segments:
            active = list(gens)
            while active:
                for item in list(active):
                    g, pulls = item
                    for _ in range(pulls):
                        try:
                            next(g)
                        except StopIteration:
                            if item in active:
                                active.remove(item)
                            break


def prepare_inputs(hidden_states, position_ids, Wqkv, Wout):
    import ml_dtypes
    bf16 = ml_dtypes.bfloat16
    hidden_states = np.asarray(hidden_states, dtype=np.float32)
    position_ids = np.asarray(position_ids)
    Wqkv = np.asarray(Wqkv, dtype=np.float32)
    Wout = np.asarray(Wout, dtype=np.float32)

    inv_freq = (1.0 / (ROPE_THETA ** (np.arange(0, HD, 2, dtype=np.float32)
                                      / np.float32(HD)))).astype(np.float32)
    in_maps = []
    for c in range(NCORES):
        b, g = divmod(c, NQ)
        pos = position_ids[b].astype(np.float32)
        freqs = pos[:, None] * inv_freq[None, :]
        cos = np.cos(np.concatenate([freqs, freqs], axis=1))
        sin = np.sin(np.concatenate([freqs, freqs], axis=1))
        sinS = sin.T.copy()
        sinS[:HD // 2] *= -1.0
        wq_rows = np.concatenate([
            Wqkv[512 * g:512 * (g + 1)],
            Wqkv[NH * HD + HD * g: NH * HD + HD * (g + 1)],
            Wqkv[(NH + NKV) * HD + HD * g:
                 (NH + NKV) * HD + HD * (g + 1)],
        ], axis=0)
        in_maps.append({
            "hT": np.ascontiguousarray(hidden_states[b].T).astype(bf16),
            "wqkvT": np.ascontiguousarray(wq_rows.T).astype(bf16),
            "woutT": np.ascontiguousarray(
                Wout[:, 512 * g:512 * (g + 1)].T).astype(bf16),
            "cosT": np.ascontiguousarray(cos.T).astype(bf16),
            "sinT": np.ascontiguousarray(sinS.astype(np.float32)),
        })
    return in_maps


def assemble(results):
    out = np.zeros((B, S, HID), dtype=np.float64)
    for c in range(NCORES):
        b = c // NQ
        out[b] += results[c]["out"].astype(np.float64)
    return out.astype(np.float32)


_cache = {}


def kernel(hidden_states, position_ids, Wqkv, Wout):
    if "nc" not in _cache:
        _cache["nc"] = build_nc(reps=1)
    nc = _cache["nc"]
    in_maps = prepare_inputs(hidden_states, position_ids, Wqkv, Wout)
    res = run_bass_kernel_spmd(nc, in_maps, core_ids=list(range(NCORES)))
    return assemble(res.results)
